# revision 39
# baseline (speedup 1.0000x reference)
"""Causal self-attention kernel for Trainium2, 8-core data parallel.

Per-core program: one batch element b of x [8, 1024, 768].

Software-pipelined structure: for each head pair p the "stretch" emits
pair p's row-tiled QK matmuls + exps (ACT engine is the pacer), weaving
between them as PE filler: the PV matmuls of pair p-1 and the Q/K
projection matmuls of pair p+1 (K psum tile first half, Q tile second
half, so only one projection tile is live at a time).  V projection
fills stretch 0; W_proj transposes fill stretch 5.  Normalization runs
entirely off the PE: ones-column denominator row -> DRAM -> broadcast
read -> DVE reciprocal_approx_fast -> DVE multiply.

PSUM (8 banks): psS ring 2x[128,512] (transients: QK S-tiles, woven
transposes / V-proj) + psP ring 3x[128,1024] (pK/pQ projection tiles,
yD PV accumulators, phase-0 transposes, out-proj).  Careful allocation
order keeps every ring-slot wait's producer earlier in program order.
All matmuls bf16 / fp32 PSUM; softmax fp32; causal P^T tiles stored
flattened (only q >= kt*128 kept).
"""
import sys
import contextlib
from contextlib import ExitStack

sys.path.insert(0, "/opt/trn_rl_repo")

import numpy as np

import concourse.bass as bass
import concourse.bacc as bacc
import concourse.mybir as mybir
import concourse.tile as tile

F32 = mybir.dt.float32
BF16 = mybir.dt.bfloat16
P = 128
T = 1024
C = 768
H = 12
HS = 64
CT = C // P
TT = T // P
KT = T // P
N_CORES = 8

# flat offsets for causal P^T storage: k-tile kt keeps q in [kt*128, 1024)
PT_OFF = [0]
for _kt in range(1, KT + 1):
    PT_OFF.append(PT_OFF[-1] + (T - 128 * (_kt - 1)))
PT_W = PT_OFF[-1]  # 4608


def qk_chunks(kt):
    qs = kt * P
    if kt < 4:
        return [(qs, 512 - qs), (512, 512)]
    return [(qs, T - qs)]


def emit_consts(nc, tc, const, ba_d, bp_d):
    from concourse.masks import make_identity
    ident = const.tile([P, P], F32, tag="ident")
    make_identity(nc, ident)
    bias_qk = const.tile([P, 12], F32, tag="bqk")
    nc.sync.dma_start(bias_qk[:], ba_d[0 : 2 * C].rearrange("(o p) -> p o", p=P))
    Bv = const.tile([P, C], F32, tag="Bv")
    nc.sync.dma_start(
        Bv[:],
        ba_d[2 * C : 3 * C].rearrange("(a j) -> a j", a=1).to_broadcast([P, C]),
    )
    Bp = const.tile([P, C], F32, tag="Bp")
    nc.sync.dma_start(
        Bp[:], bp_d.rearrange("(a j) -> a j", a=1).to_broadcast([P, C])
    )
    return dict(ident=ident, bias_qk=bias_qk, Bv=Bv, Bp=Bp)


def emit_body(nc, tc, pools, cst, x_d, wa_d, wp_d, y_d):
    const, persist, nat, work, ptp, dram, psS, psP = pools
    bias_qk, Bv, Bp = cst["bias_qk"], cst["Bv"], cst["Bp"]
    ident = cst["ident"]

    xT = persist.tile([P, CT, T], BF16, tag="xT")
    WT = persist.tile([P, CT, 3 * C], BF16, tag="WT")
    WpT = persist.tile([P, CT, C], BF16, tag="WpT")
    qT = persist.tile([P, 6, T], BF16, tag="qT")
    KTp = persist.tile([P, 6, T], BF16, tag="KTp")
    V = persist.tile([P, TT, 12, 65], BF16, tag="Vaug")
    yT = persist.tile([P, CT, T], BF16, tag="yT")
    osb_acc = persist.tile([P, TT, C], F32, tag="osbacc")

    # ---- direct DMA loads of pre-transposed bf16 inputs
    def load_ct(dst, dram_srcT, ct, j0, j1):
        # dst[:, ct, j0:j1] <- dram_srcT[ct*128:(ct+1)*128, j0:j1]
        nc.sync.dma_start(
            dst[:, ct, j0:j1],
            dram_srcT.rearrange("(ct p) j -> ct p j", p=P)[ct][:, j0:j1],
        )

    # ---- Q/K projection pieces (one PSUM tile at a time: K then Q)
    def proj_alloc(name):
        return psP.tile([P, 1024], F32, tag="pp", name=name)

    def proj_mms(jt, pt):
        thunks = []
        for tb in range(2):
            for ct in range(CT):
                def mm(jt=jt, pt=pt, tb=tb, ct=ct):
                    nc.tensor.matmul(
                        pt[:, tb * 512 : (tb + 1) * 512],
                        WT[:, ct, jt * P : (jt + 1) * P],
                        xT[:, ct, tb * 512 : (tb + 1) * 512],
                        start=(ct == 0),
                        stop=(ct == CT - 1),
                    )
                thunks.append(mm)
        return thunks

    def proj_copy(j, jt, pt):
        dst = qT if jt < 6 else KTp
        nc.vector.tensor_scalar_add(dst[:, j, :], pt[:], bias_qk[:, jt : jt + 1])

    # ---- V projection for one tt (psS transient tiles, one jb at a time)
    def vproj_thunk(tt):
        def go():
            for jb in range(2):
                ps = psS.tile([P, 512], F32, tag="sps", name="vps")
                for ct in range(CT):
                    nc.tensor.matmul(
                        ps[:, :384],
                        xT[:, ct, tt * P : (tt + 1) * P],
                        WT[:, ct, 2 * C + jb * 384 : 2 * C + (jb + 1) * 384],
                        start=(ct == 0),
                        stop=(ct == CT - 1),
                    )
                sl = slice(jb * 384, (jb + 1) * 384)
                nc.vector.tensor_add(
                    V[:, tt, 6 * jb : 6 * jb + 6, 0:64],
                    ps[:, :384].rearrange("p (h d) -> p h d", d=64),
                    Bv[:, sl].rearrange("p (h d) -> p h d", d=64),
                )
        return go

    # ---- PV matmul thunks (single-MM granularity for smooth weaving)
    def pv_thunks(h, PT, yD, blocks=(0, 1)):
        thunks = []
        for b in blocks:
            for kt in range(KT):
                if b == 0 and kt >= 4:
                    continue
                off = max(0, kt * P - b * 512)
                w = 512 - off
                first = kt == 0
                last = kt == (3 if b == 0 else KT - 1)
                q0 = b * 512 + off
                fo = PT_OFF[kt] + (q0 - kt * P)
                def mm(h=h, PT=PT, yD=yD, q0=q0, w=w, fo=fo,
                       first=first, last=last, kt=kt):
                    nc.tensor.matmul(
                        yD[0:65, q0 : q0 + w],
                        V[:, kt, h, :],
                        PT[:, fo : fo + w],
                        start=first,
                        stop=last,
                    )
                thunks.append(mm)
        return thunks

    def emit_norm_head(h, yD, b0=0, b1=2):
        n = (b1 - b0) * 512
        sl = slice(b0 * 512, b1 * 512)
        Dsb = work.tile([65, T], F32, tag="Dsb")
        nc.vector.tensor_copy(Dsb[64:65, sl], yD[64:65, sl])
        Dd = dram.tile([T], F32, tag="Dd")
        nc.sync.dma_start(Dd[sl].rearrange("(a t) -> a t", a=1), Dsb[64:65, sl])
        Dfull = work.tile([64, T], F32, tag="Dfull")
        nc.sync.dma_start(
            Dfull[0:64, sl],
            Dd[sl].rearrange("(a t) -> a t", a=1).to_broadcast([64, n]),
        )
        Rh = work.tile([64, T], F32, tag="Rh")
        nc.vector.reciprocal_approx_fast(Rh[0:64, sl], Dfull[0:64, sl])
        if h % 2 == 0:
            nc.vector.tensor_mul(yT[0:64, h // 2, sl], yD[0:64, sl], Rh[0:64, sl])
        else:
            yTt = work.tile([64, T], BF16, tag="yTt")
            nc.vector.tensor_mul(yTt[0:64, sl], yD[0:64, sl], Rh[0:64, sl])
            nc.sync.dma_start(yT[64:128, h // 2, sl], yTt[0:64, sl])

    def outproj_partial_thunk(tt):
        # accumulate ct 0-3 (heads 0-7) + bias into the SBUF accumulator
        def go():
            pss = [psS.tile([P, 512], F32, tag="sps", name="opp") for _ in range(2)]
            for ct in range(4):
                for jb in range(2):
                    nc.tensor.matmul(
                        pss[jb][:, :384],
                        yT[:, ct, tt * P : (tt + 1) * P],
                        WpT[:, ct, jb * 384 : (jb + 1) * 384],
                        start=(ct == 0),
                        stop=(ct == 3),
                    )
            for jb in range(2):
                sl = slice(jb * 384, (jb + 1) * 384)
                nc.vector.tensor_add(
                    osb_acc[:, tt, sl], pss[jb][:, :384], Bp[:, sl]
                )
        return go

    def emit_outproj_finish(tt):
        pss = [psS.tile([P, 512], F32, tag="sps", name="opf") for _ in range(2)]
        for ct in (4, 5):
            for jb in range(2):
                nc.tensor.matmul(
                    pss[jb][:, :384],
                    yT[:, ct, tt * P : (tt + 1) * P],
                    WpT[:, ct, jb * 384 : (jb + 1) * 384],
                    start=(ct == 4),
                    stop=(ct == 5),
                )
        osb = work.tile([P, C], F32, tag="osb")
        for jb in range(2):
            sl = slice(jb * 384, (jb + 1) * 384)
            nc.vector.tensor_add(
                osb[:, sl], pss[jb][:, :384], osb_acc[:, tt, sl]
            )
        nc.sync.dma_start(y_d.rearrange("(tt p) c -> tt p c", p=P)[tt], osb[:])

    def emit_outproj(tt):
        pss = psP.tile([P, 1024], F32, tag="pp", name="ops")
        for ct in range(CT):
            for jb in range(2):
                nc.tensor.matmul(
                    pss[:, jb * 512 : jb * 512 + 384],
                    yT[:, ct, tt * P : (tt + 1) * P],
                    WpT[:, ct, jb * 384 : (jb + 1) * 384],
                    start=(ct == 0),
                    stop=(ct == CT - 1),
                )
        osb = work.tile([P, C], F32, tag="osb")
        for jb in range(2):
            sl = slice(jb * 384, (jb + 1) * 384)
            nc.vector.tensor_add(osb[:, sl], pss[:, jb * 512 : jb * 512 + 384], Bp[:, sl])
        nc.sync.dma_start(y_d.rearrange("(tt p) c -> tt p c", p=P)[tt], osb[:])

    def weave2(a, b):
        """Evenly interleave two thunk lists."""
        out = []
        ia = ib = 0
        n = len(a) + len(b)
        for k in range(1, n + 1):
            wa = (len(a) * k) // n
            while ia < wa:
                out.append(a[ia]); ia += 1
            wb = (len(b) * k) // n
            while ib < wb:
                out.append(b[ib]); ib += 1
        return out

    # ---- the QK stretch for pair j with first/second-half filler queues
    def emit_stretch(j, first_half, second_half, late=(), holder=None):
        PTa = ptp.tile([P, PT_W], BF16, tag="PT", name=f"PTa{j}")
        PTb = ptp.tile([P, PT_W], BF16, tag="PT", name=f"PTb{j}")
        if holder is not None:
            holder["PTa"], holder["PTb"] = PTa, PTb
        points = sum(len(qk_chunks(kt)) for kt in range(KT))  # 12
        half_pt = points // 2
        pi = 0
        f1 = f2 = 0
        li = 0
        for kt in range(KT):
            qs = kt * P
            for ci, (q0, w) in enumerate(qk_chunks(kt)):
                spsa = psS.tile([P, 512], F32, tag="sps", name="spsa")
                spsb = psS.tile([P, 512], F32, tag="sps", name="spsb")
                nc.tensor.matmul(
                    spsa[:, :w],
                    KTp[0:64, j, kt * P : (kt + 1) * P],
                    qT[0:64, j, q0 : q0 + w],
                    start=True, stop=True,
                    tile_position=(0, 0),
                )
                nc.tensor.matmul(
                    spsb[:, :w],
                    KTp[64:128, j, kt * P : (kt + 1) * P],
                    qT[64:128, j, q0 : q0 + w],
                    start=True, stop=True,
                    tile_position=(64, 0),
                )
                fo = PT_OFF[kt] + (q0 - qs)
                nc.scalar.activation(
                    PTa[:, fo : fo + w], spsa[:, :w],
                    mybir.ActivationFunctionType.Exp, scale=0.125,
                )
                nc.scalar.activation(
                    PTb[:, fo : fo + w], spsb[:, :w],
                    mybir.ActivationFunctionType.Exp, scale=0.125,
                )
                if ci == 0:
                    dg = PT_OFF[kt]
                    for PT in (PTa, PTb):
                        nc.gpsimd.affine_select(
                            out=PT[:, dg : dg + P],
                            in_=PT[:, dg : dg + P],
                            compare_op=mybir.AluOpType.is_ge,
                            fill=0.0,
                            base=0,
                            pattern=[[1, P]],
                            channel_multiplier=-1,
                        )
                pi += 1
                if pi <= half_pt:
                    want = (len(first_half) * pi) // half_pt
                    while f1 < want:
                        first_half[f1]()
                        f1 += 1
                else:
                    want = (len(second_half) * (pi - half_pt)) // (points - half_pt)
                    while f2 < want:
                        second_half[f2]()
                        f2 += 1
            while li < len(late) and late[li][0] <= kt:
                late[li][1]()
                li += 1
        while f1 < len(first_half):
            first_half[f1]()
            f1 += 1
        while f2 < len(second_half):
            second_half[f2]()
            f2 += 1
        while li < len(late):
            late[li][1]()
            li += 1
        return PTa, PTb

    # ================= prologue: DMA loads + pair-0 projection =================
    # interleave x and Q/K-weight loads per ct so proj0's accumulation can
    # chase the DMA stream; V columns and W_proj follow.
    for ct in range(CT):
        load_ct(xT, x_d, ct, 0, T)
        load_ct(WT, wa_d, ct, 0, 2 * C)
    for ct in range(CT):
        load_ct(WT, wa_d, ct, 2 * C, 3 * C)      # V columns
    for ct in range(CT):
        load_ct(WpT, wp_d, ct, 0, C)
    pK0 = proj_alloc("pK0")
    for mm in proj_mms(6, pK0):
        mm()
    proj_copy(0, 6, pK0)
    pQ0 = proj_alloc("pQ0")
    for mm in proj_mms(0, pQ0):
        mm()
    proj_copy(0, 0, pQ0)

    # ================= pair loop =================
    PTs = {}
    for p in range(6):
        if p == 0:
            # stretch 0: V projection (psS transients) + pair-1 projections
            pK = proj_alloc("pK1")
            projK = proj_mms(7, pK)
            first = weave2([vproj_thunk(i) for i in range(4)], projK)
            holder = {}
            def mid0(pK=pK):
                proj_copy(1, 7, pK)
                holder["pQ"] = proj_alloc("pQ1")
                holder["q"] = proj_mms(1, holder["pQ"])
            second = [mid0]
            second += weave2(
                [vproj_thunk(i) for i in range(4, TT)],
                [lambda k=k: holder["q"][k]() for k in range(12)],
            )
            second.append(lambda: proj_copy(1, 1, holder["pQ"]))
            PTa, PTb = emit_stretch(0, first, second)
            PTs[0], PTs[1] = PTa, PTb
            continue
        ha, hb = 2 * p - 2, 2 * p - 1
        PTa_, PTb_ = PTs.pop(ha), PTs.pop(hb)
        if p < 5:
            jn = p + 1
            pK = proj_alloc(f"pK{jn}")
            yDa = psP.tile([P, 1024], F32, tag="pp", name=f"yD{ha}")
            projK = proj_mms(6 + jn, pK)
            pva = pv_thunks(ha, PTa_, yDa)
            first = weave2(projK, pva)
            holder = {}
            def mid(jn=jn, pK=pK, ha=ha, yDa=yDa):
                proj_copy(jn, 6 + jn, pK)
                holder["pQ"] = proj_alloc(f"pQ{jn}")
                holder["yDb"] = psP.tile([P, 1024], F32, tag="pp",
                                         name=f"yD{ha+1}")
                holder["q"] = proj_mms(jn, holder["pQ"])
                holder["pvb"] = pv_thunks(ha + 1, PTb_, holder["yDb"])
                emit_norm_head(ha, yDa)
            second = [mid]
            second += weave2(
                [lambda k=k: holder["q"][k]() for k in range(12)],
                [lambda k=k: holder["pvb"][k]() for k in range(12)],
            )
            def tail(jn=jn, hb=hb):
                proj_copy(jn, jn, holder["pQ"])
                emit_norm_head(hb, holder["yDb"])
            second.append(tail)
            PTa, PTb = emit_stretch(p, first, second)
            PTs[2 * p], PTs[2 * p + 1] = PTa, PTb
        else:
            # stretch 5: W_proj transposes (psS transients) + PV(pair 4)
            yDa = psP.tile([P, 1024], F32, tag="pp", name="yD8")
            pva = pv_thunks(8, PTa_, yDa)
            first = weave2(pva, [outproj_partial_thunk(tt) for tt in range(4)])
            holder = {}
            def mid5(yDa=yDa):
                emit_norm_head(8, yDa)
                holder["yDb"] = psP.tile([P, 1024], F32, tag="pp", name="yD9")
                holder["pvb"] = pv_thunks(9, PTb_, holder["yDb"])
            second = [mid5]
            second += weave2(
                [lambda k=k: holder["pvb"][k]() for k in range(12)],
                [outproj_partial_thunk(tt) for tt in range(4, TT)],
            )
            second.append(lambda: emit_norm_head(9, holder["yDb"]))
            # weave pair-5's b=0 PV + norms into the stretch once their
            # k-tiles (0-3) are exp'd, so the norm chains drain in-stretch
            h2 = {}
            def late_10():
                h2["yD10"] = psP.tile([P, 1024], F32, tag="pp", name="yD10")
                for t in pv_thunks(10, h2["PTa"], h2["yD10"], blocks=(0,)):
                    t()
                emit_norm_head(10, h2["yD10"], 0, 1)
            def late_11():
                h2["yD11"] = psP.tile([P, 1024], F32, tag="pp", name="yD11")
                for t in pv_thunks(11, h2["PTb"], h2["yD11"], blocks=(0,)):
                    t()
                emit_norm_head(11, h2["yD11"], 0, 1)
            late = [(3, late_10), (4, late_11)]
            PTa, PTb = emit_stretch(5, first, second, late=late, holder=h2)
            # epilogue: b=1 halves, then out-proj (tt 0-3 gated only by the
            # already-drained b=0 norms)
            for t in pv_thunks(10, PTa, h2["yD10"], blocks=(1,)):
                t()
            for t in pv_thunks(11, PTb, h2["yD11"], blocks=(1,)):
                t()
            emit_norm_head(10, h2["yD10"], 1, 2)
            emit_norm_head(11, h2["yD11"], 1, 2)
            for tt in range(TT):
                emit_outproj_finish(tt)


def build_program(loop=1):
    nc = bacc.Bacc("TRN2", target_bir_lowering=False, debug=False)
    x_d = nc.dram_tensor("xT", [C, T], BF16, kind="ExternalInput").ap()
    wa_d = nc.dram_tensor("WaT", [C, 3 * C], BF16, kind="ExternalInput").ap()
    ba_d = nc.dram_tensor("b_attn", [3 * C], F32, kind="ExternalInput").ap()
    wp_d = nc.dram_tensor("WpT", [C, C], BF16, kind="ExternalInput").ap()
    bp_d = nc.dram_tensor("b_proj", [C], F32, kind="ExternalInput").ap()
    y_d = nc.dram_tensor("y", [T, C], F32, kind="ExternalOutput").ap()

    with tile.TileContext(nc) as tc, ExitStack() as ctx:
        const = ctx.enter_context(tc.tile_pool(name="const", bufs=1))
        persist = ctx.enter_context(tc.tile_pool(name="persist", bufs=1))
        nat = ctx.enter_context(tc.tile_pool(name="nat", bufs=6))
        work = ctx.enter_context(tc.tile_pool(name="work", bufs=2))
        ptp = ctx.enter_context(tc.tile_pool(name="ptp", bufs=4))
        dram = ctx.enter_context(tc.tile_pool(name="dram", bufs=2, space="DRAM"))
        psS = ctx.enter_context(tc.tile_pool(name="psS", bufs=2, space="PSUM"))
        psP = ctx.enter_context(tc.tile_pool(name="psP", bufs=3, space="PSUM"))
        pools = (const, persist, nat, work, ptp, dram, psS, psP)

        cst = emit_consts(nc, tc, const, ba_d, bp_d)
        V0 = persist.tile([P, TT, 12, 65], BF16, tag="Vaug")
        nc.gpsimd.memset(V0[:, :, :, 64:65], 1.0)
        loop_cm = tc.For_i(0, loop, 1) if loop > 1 else contextlib.nullcontext()
        with loop_cm:
            emit_body(nc, tc, pools, cst, x_d, wa_d, wp_d, y_d)

    nc.compile()
    return nc


_CACHED_NC = None


def prep_in_maps(x, W_attn, b_attn, W_proj, b_proj):
    import ml_dtypes
    bf16 = ml_dtypes.bfloat16
    B = x.shape[0]
    assert B == N_CORES
    WaT = np.ascontiguousarray(np.asarray(W_attn, dtype=np.float32).T.astype(bf16))
    WpT = np.ascontiguousarray(np.asarray(W_proj, dtype=np.float32).T.astype(bf16))
    ba = np.asarray(b_attn, dtype=np.float32)
    bp = np.asarray(b_proj, dtype=np.float32)
    return [
        {
            "xT": np.ascontiguousarray(
                np.asarray(x[b], dtype=np.float32).T.astype(bf16)),
            "WaT": WaT,
            "b_attn": ba,
            "WpT": WpT,
            "b_proj": bp,
        }
        for b in range(B)
    ]


def kernel(x, W_attn, b_attn, W_proj, b_proj):
    from concourse.bass_utils import run_bass_kernel_spmd

    global _CACHED_NC
    if _CACHED_NC is None:
        _CACHED_NC = build_program(loop=1)
    nc = _CACHED_NC

    in_maps = prep_in_maps(x, W_attn, b_attn, W_proj, b_proj)
    res = run_bass_kernel_spmd(nc, in_maps, list(range(N_CORES)))
    return np.stack([res.results[b]["y"] for b in range(N_CORES)], axis=0)


# revision 40
# speedup vs baseline: 1.1803x; 1.1803x over previous
"""Causal self-attention kernel for Trainium2, 8-core data parallel.

Per-core program: one batch element b of x [8, 1024, 768].

Software-pipelined structure: for each head pair p the "stretch" emits
pair p's row-tiled QK matmuls + exps (ACT engine is the pacer), weaving
between them as PE filler: the PV matmuls of pair p-1 and the Q/K
projection matmuls of pair p+1 (K psum tile first half, Q tile second
half, so only one projection tile is live at a time).  V projection
fills stretch 0; W_proj transposes fill stretch 5.  Normalization runs
entirely off the PE: ones-column denominator row -> DRAM -> broadcast
read -> DVE reciprocal_approx_fast -> DVE multiply.

PSUM (8 banks): psS ring 2x[128,512] (transients: QK S-tiles, woven
transposes / V-proj) + psP ring 3x[128,1024] (pK/pQ projection tiles,
yD PV accumulators, phase-0 transposes, out-proj).  Careful allocation
order keeps every ring-slot wait's producer earlier in program order.
All matmuls bf16 / fp32 PSUM; softmax fp32; causal P^T tiles stored
flattened (only q >= kt*128 kept).
"""
import sys
import contextlib
from contextlib import ExitStack

sys.path.insert(0, "/opt/trn_rl_repo")

import numpy as np

import concourse.bass as bass
import concourse.bacc as bacc
import concourse.mybir as mybir
import concourse.tile as tile

F32 = mybir.dt.float32
BF16 = mybir.dt.bfloat16
P = 128
T = 1024
C = 768
H = 12
HS = 64
CT = C // P
TT = T // P
KT = T // P
N_CORES = 8

# flat offsets for causal P^T storage: k-tile kt keeps q in [kt*128, 1024)
PT_OFF = [0]
for _kt in range(1, KT + 1):
    PT_OFF.append(PT_OFF[-1] + (T - 128 * (_kt - 1)))
PT_W = PT_OFF[-1]  # 4608


def qk_chunks(kt):
    qs = kt * P
    if kt < 4:
        return [(qs, 512 - qs), (512, 512)]
    return [(qs, T - qs)]


def emit_consts(nc, tc, const, ba_d, bp_d):
    from concourse.masks import make_identity
    ident = const.tile([P, P], F32, tag="ident")
    make_identity(nc, ident)
    bias_qk = const.tile([P, 12], F32, tag="bqk")
    nc.sync.dma_start(bias_qk[:], ba_d[0 : 2 * C].rearrange("(o p) -> p o", p=P))
    Bv = const.tile([P, C], F32, tag="Bv")
    nc.sync.dma_start(
        Bv[:],
        ba_d[2 * C : 3 * C].rearrange("(a j) -> a j", a=1).to_broadcast([P, C]),
    )
    Bp = const.tile([P, C], F32, tag="Bp")
    nc.sync.dma_start(
        Bp[:], bp_d.rearrange("(a j) -> a j", a=1).to_broadcast([P, C])
    )
    return dict(ident=ident, bias_qk=bias_qk, Bv=Bv, Bp=Bp)


def emit_body(nc, tc, pools, cst, x_d, wa_d, wp_d, y_d):
    const, persist, nat, work, ptp, dram, psS, psP = pools
    bias_qk, Bv, Bp = cst["bias_qk"], cst["Bv"], cst["Bp"]
    ident = cst["ident"]

    xT = persist.tile([P, CT, T], BF16, tag="xT")
    WT = persist.tile([P, CT, 3 * C], BF16, tag="WT")
    WpT = persist.tile([P, CT, C], BF16, tag="WpT")
    qT = persist.tile([P, 6, T], BF16, tag="qT")
    KTp = persist.tile([P, 6, T], BF16, tag="KTp")
    V = persist.tile([P, TT, 12, 65], BF16, tag="Vaug")
    yT = persist.tile([P, CT, T], BF16, tag="yT")
    osb_acc = persist.tile([P, TT, C], F32, tag="osbacc")

    # ---- direct DMA loads of pre-transposed bf16 inputs
    def load_ct(dst, dram_srcT, ct, j0, j1):
        # dst[:, ct, j0:j1] <- dram_srcT[ct*128:(ct+1)*128, j0:j1]
        nc.sync.dma_start(
            dst[:, ct, j0:j1],
            dram_srcT.rearrange("(ct p) j -> ct p j", p=P)[ct][:, j0:j1],
        )

    # ---- Q/K projection pieces (one PSUM tile at a time: K then Q)
    def proj_alloc(name):
        return psP.tile([P, 1024], F32, tag="pp", name=name)

    def proj_mms(jt, pt):
        thunks = []
        for tb in range(2):
            for ct in range(CT):
                def mm(jt=jt, pt=pt, tb=tb, ct=ct):
                    nc.tensor.matmul(
                        pt[:, tb * 512 : (tb + 1) * 512],
                        WT[:, ct, jt * P : (jt + 1) * P],
                        xT[:, ct, tb * 512 : (tb + 1) * 512],
                        start=(ct == 0),
                        stop=(ct == CT - 1),
                    )
                thunks.append(mm)
        return thunks

    def proj_copy(j, jt, pt):
        dst = qT if jt < 6 else KTp
        nc.vector.tensor_scalar_add(dst[:, j, :], pt[:], bias_qk[:, jt : jt + 1])

    # ---- V projection for one tt (psS transient tiles, one jb at a time)
    def vproj_thunk(tt):
        def go():
            for jb in range(2):
                ps = psS.tile([P, 512], F32, tag="sps", name="vps")
                for ct in range(CT):
                    nc.tensor.matmul(
                        ps[:, :384],
                        xT[:, ct, tt * P : (tt + 1) * P],
                        WT[:, ct, 2 * C + jb * 384 : 2 * C + (jb + 1) * 384],
                        start=(ct == 0),
                        stop=(ct == CT - 1),
                    )
                sl = slice(jb * 384, (jb + 1) * 384)
                nc.vector.tensor_add(
                    V[:, tt, 6 * jb : 6 * jb + 6, 0:64],
                    ps[:, :384].rearrange("p (h d) -> p h d", d=64),
                    Bv[:, sl].rearrange("p (h d) -> p h d", d=64),
                )
        return go

    # ---- PV matmul thunks (single-MM granularity for smooth weaving)
    def pv_thunks(h, PT, yD, blocks=(0, 1)):
        thunks = []
        for b in blocks:
            for kt in range(KT):
                if b == 0 and kt >= 4:
                    continue
                off = max(0, kt * P - b * 512)
                w = 512 - off
                first = kt == 0
                last = kt == (3 if b == 0 else KT - 1)
                q0 = b * 512 + off
                fo = PT_OFF[kt] + (q0 - kt * P)
                def mm(h=h, PT=PT, yD=yD, q0=q0, w=w, fo=fo,
                       first=first, last=last, kt=kt):
                    nc.tensor.matmul(
                        yD[0:65, q0 : q0 + w],
                        V[:, kt, h, :],
                        PT[:, fo : fo + w],
                        start=first,
                        stop=last,
                    )
                thunks.append(mm)
        return thunks

    def emit_norm_head(h, yD, b0=0, b1=2):
        n = (b1 - b0) * 512
        sl = slice(b0 * 512, b1 * 512)
        Dsb = work.tile([65, T], F32, tag="Dsb")
        nc.vector.tensor_copy(Dsb[64:65, sl], yD[64:65, sl])
        Dd = dram.tile([T], F32, tag="Dd")
        nc.sync.dma_start(Dd[sl].rearrange("(a t) -> a t", a=1), Dsb[64:65, sl])
        Dfull = work.tile([64, T], F32, tag="Dfull")
        nc.sync.dma_start(
            Dfull[0:64, sl],
            Dd[sl].rearrange("(a t) -> a t", a=1).to_broadcast([64, n]),
        )
        Rh = work.tile([64, T], F32, tag="Rh")
        nc.vector.reciprocal_approx_fast(Rh[0:64, sl], Dfull[0:64, sl])
        if h % 2 == 0:
            nc.vector.tensor_mul(yT[0:64, h // 2, sl], yD[0:64, sl], Rh[0:64, sl])
        else:
            yTt = work.tile([64, T], BF16, tag="yTt")
            nc.vector.tensor_mul(yTt[0:64, sl], yD[0:64, sl], Rh[0:64, sl])
            nc.sync.dma_start(yT[64:128, h // 2, sl], yTt[0:64, sl])

    def outproj_partial_thunk(tt, jb):
        # accumulate ct 0-3 (heads 0-7) + bias into the SBUF accumulator;
        # one psS tile per thunk so the QK/exp stream is never starved
        def go():
            ps = psS.tile([P, 512], F32, tag="sps", name="opp")
            for ct in range(4):
                nc.tensor.matmul(
                    ps[:, :384],
                    yT[:, ct, tt * P : (tt + 1) * P],
                    WpT[:, ct, jb * 384 : (jb + 1) * 384],
                    start=(ct == 0),
                    stop=(ct == 3),
                )
            sl = slice(jb * 384, (jb + 1) * 384)
            nc.vector.tensor_add(osb_acc[:, tt, sl], ps[:, :384], Bp[:, sl])
        return go

    def outproj_finish_thunk(tt, jb):
        def go():
            ps = psS.tile([P, 512], F32, tag="sps", name="opf")
            for ct in (4, 5):
                nc.tensor.matmul(
                    ps[:, :384],
                    yT[:, ct, tt * P : (tt + 1) * P],
                    WpT[:, ct, jb * 384 : (jb + 1) * 384],
                    start=(ct == 4),
                    stop=(ct == 5),
                )
            sl = slice(jb * 384, (jb + 1) * 384)
            osb = osb_acc  # reuse accumulator as the staging buffer
            nc.vector.tensor_add(osb[:, tt, sl], ps[:, :384], osb_acc[:, tt, sl])
        return go

    def emit_out_dma(tt):
        nc.sync.dma_start(
            y_d.rearrange("(tt p) c -> tt p c", p=P)[tt], osb_acc[:, tt, :]
        )

    def emit_outproj(tt):
        pss = psP.tile([P, 1024], F32, tag="pp", name="ops")
        for ct in range(CT):
            for jb in range(2):
                nc.tensor.matmul(
                    pss[:, jb * 512 : jb * 512 + 384],
                    yT[:, ct, tt * P : (tt + 1) * P],
                    WpT[:, ct, jb * 384 : (jb + 1) * 384],
                    start=(ct == 0),
                    stop=(ct == CT - 1),
                )
        osb = work.tile([P, C], F32, tag="osb")
        for jb in range(2):
            sl = slice(jb * 384, (jb + 1) * 384)
            nc.vector.tensor_add(osb[:, sl], pss[:, jb * 512 : jb * 512 + 384], Bp[:, sl])
        nc.sync.dma_start(y_d.rearrange("(tt p) c -> tt p c", p=P)[tt], osb[:])

    def weave2(a, b):
        """Evenly interleave two thunk lists."""
        out = []
        ia = ib = 0
        n = len(a) + len(b)
        for k in range(1, n + 1):
            wa = (len(a) * k) // n
            while ia < wa:
                out.append(a[ia]); ia += 1
            wb = (len(b) * k) // n
            while ib < wb:
                out.append(b[ib]); ib += 1
        return out

    # ---- the QK stretch for pair j with first/second-half filler queues
    def emit_stretch(j, first_half, second_half, late=(), holder=None):
        PTa = ptp.tile([P, PT_W], BF16, tag="PT", name=f"PTa{j}")
        PTb = ptp.tile([P, PT_W], BF16, tag="PT", name=f"PTb{j}")
        if holder is not None:
            holder["PTa"], holder["PTb"] = PTa, PTb
        points = sum(len(qk_chunks(kt)) for kt in range(KT))  # 12
        half_pt = points // 2
        pi = 0
        f1 = f2 = 0
        li = 0
        for kt in range(KT):
            qs = kt * P
            for ci, (q0, w) in enumerate(qk_chunks(kt)):
                spsa = psS.tile([P, 512], F32, tag="sps", name="spsa")
                spsb = psS.tile([P, 512], F32, tag="sps", name="spsb")
                nc.tensor.matmul(
                    spsa[:, :w],
                    KTp[0:64, j, kt * P : (kt + 1) * P],
                    qT[0:64, j, q0 : q0 + w],
                    start=True, stop=True,
                    tile_position=(0, 0),
                )
                nc.tensor.matmul(
                    spsb[:, :w],
                    KTp[64:128, j, kt * P : (kt + 1) * P],
                    qT[64:128, j, q0 : q0 + w],
                    start=True, stop=True,
                    tile_position=(64, 0),
                )
                fo = PT_OFF[kt] + (q0 - qs)
                nc.scalar.activation(
                    PTa[:, fo : fo + w], spsa[:, :w],
                    mybir.ActivationFunctionType.Exp, scale=0.125,
                )
                nc.scalar.activation(
                    PTb[:, fo : fo + w], spsb[:, :w],
                    mybir.ActivationFunctionType.Exp, scale=0.125,
                )
                if ci == 0:
                    dg = PT_OFF[kt]
                    for PT in (PTa, PTb):
                        nc.gpsimd.affine_select(
                            out=PT[:, dg : dg + P],
                            in_=PT[:, dg : dg + P],
                            compare_op=mybir.AluOpType.is_ge,
                            fill=0.0,
                            base=0,
                            pattern=[[1, P]],
                            channel_multiplier=-1,
                        )
                pi += 1
                if pi <= half_pt:
                    want = (len(first_half) * pi) // half_pt
                    while f1 < want:
                        first_half[f1]()
                        f1 += 1
                else:
                    want = (len(second_half) * (pi - half_pt)) // (points - half_pt)
                    while f2 < want:
                        second_half[f2]()
                        f2 += 1
            while li < len(late) and late[li][0] <= kt:
                late[li][1]()
                li += 1
        while f1 < len(first_half):
            first_half[f1]()
            f1 += 1
        while f2 < len(second_half):
            second_half[f2]()
            f2 += 1
        while li < len(late):
            late[li][1]()
            li += 1
        return PTa, PTb

    # ================= prologue: DMA loads + pair-0 projection =================
    # interleave x and Q/K-weight loads per ct so proj0's accumulation can
    # chase the DMA stream; V columns and W_proj follow.
    for ct in range(CT):
        load_ct(xT, x_d, ct, 0, T)
        load_ct(WT, wa_d, ct, 0, 2 * C)
    for ct in range(CT):
        load_ct(WT, wa_d, ct, 2 * C, 3 * C)      # V columns
    for ct in range(CT):
        load_ct(WpT, wp_d, ct, 0, C)
    pK0 = proj_alloc("pK0")
    for mm in proj_mms(6, pK0):
        mm()
    proj_copy(0, 6, pK0)
    pQ0 = proj_alloc("pQ0")
    for mm in proj_mms(0, pQ0):
        mm()
    proj_copy(0, 0, pQ0)

    # ================= pair loop =================
    PTs = {}
    for p in range(6):
        if p == 0:
            # stretch 0: V projection (psS transients) + pair-1 projections
            pK = proj_alloc("pK1")
            projK = proj_mms(7, pK)
            first = weave2([vproj_thunk(i) for i in range(4)], projK)
            holder = {}
            def mid0(pK=pK):
                proj_copy(1, 7, pK)
                holder["pQ"] = proj_alloc("pQ1")
                holder["q"] = proj_mms(1, holder["pQ"])
            second = [mid0]
            second += weave2(
                [vproj_thunk(i) for i in range(4, TT)],
                [lambda k=k: holder["q"][k]() for k in range(12)],
            )
            second.append(lambda: proj_copy(1, 1, holder["pQ"]))
            PTa, PTb = emit_stretch(0, first, second)
            PTs[0], PTs[1] = PTa, PTb
            continue
        ha, hb = 2 * p - 2, 2 * p - 1
        PTa_, PTb_ = PTs.pop(ha), PTs.pop(hb)
        if p < 5:
            jn = p + 1
            pK = proj_alloc(f"pK{jn}")
            yDa = psP.tile([P, 1024], F32, tag="pp", name=f"yD{ha}")
            projK = proj_mms(6 + jn, pK)
            pva = pv_thunks(ha, PTa_, yDa)
            first = weave2(projK, pva)
            holder = {}
            def mid(jn=jn, pK=pK, ha=ha, yDa=yDa):
                proj_copy(jn, 6 + jn, pK)
                holder["pQ"] = proj_alloc(f"pQ{jn}")
                holder["yDb"] = psP.tile([P, 1024], F32, tag="pp",
                                         name=f"yD{ha+1}")
                holder["q"] = proj_mms(jn, holder["pQ"])
                holder["pvb"] = pv_thunks(ha + 1, PTb_, holder["yDb"])
                emit_norm_head(ha, yDa)
            second = [mid]
            second += weave2(
                [lambda k=k: holder["q"][k]() for k in range(12)],
                [lambda k=k: holder["pvb"][k]() for k in range(12)],
            )
            def tail(jn=jn, hb=hb):
                proj_copy(jn, jn, holder["pQ"])
                emit_norm_head(hb, holder["yDb"])
            second.append(tail)
            PTa, PTb = emit_stretch(p, first, second)
            PTs[2 * p], PTs[2 * p + 1] = PTa, PTb
        else:
            # stretch 5: W_proj transposes (psS transients) + PV(pair 4)
            yDa = psP.tile([P, 1024], F32, tag="pp", name="yD8")
            pva = pv_thunks(8, PTa_, yDa)
            first = weave2(pva, [outproj_partial_thunk(tt, jb)
                                 for tt in range(4) for jb in range(2)])
            holder = {}
            def mid5(yDa=yDa):
                emit_norm_head(8, yDa)
                holder["yDb"] = psP.tile([P, 1024], F32, tag="pp", name="yD9")
                holder["pvb"] = pv_thunks(9, PTb_, holder["yDb"])
            second = [mid5]
            second += weave2(
                [lambda k=k: holder["pvb"][k]() for k in range(12)],
                [outproj_partial_thunk(tt, jb) for tt in range(4, TT) for jb in range(2)],
            )
            second.append(lambda: emit_norm_head(9, holder["yDb"]))
            # weave pair-5's b=0 PV + norms into the stretch once their
            # k-tiles (0-3) are exp'd, so the norm chains drain in-stretch
            h2 = {}
            def late_10():
                h2["yD10"] = psP.tile([P, 1024], F32, tag="pp", name="yD10")
                for t in pv_thunks(10, h2["PTa"], h2["yD10"], blocks=(0,)):
                    t()
                emit_norm_head(10, h2["yD10"], 0, 1)
            def late_11():
                h2["yD11"] = psP.tile([P, 1024], F32, tag="pp", name="yD11")
                for t in pv_thunks(11, h2["PTb"], h2["yD11"], blocks=(0,)):
                    t()
                emit_norm_head(11, h2["yD11"], 0, 1)
            late = [(3, late_10), (4, late_11)]
            PTa, PTb = emit_stretch(5, first, second, late=late, holder=h2)
            # epilogue: b=1 halves; finishes for tt 0-3 (gated by the
            # already-drained b=0 norms) interleave behind the PVs
            for t in pv_thunks(10, PTa, h2["yD10"], blocks=(1,)):
                t()
            for t in pv_thunks(11, PTb, h2["yD11"], blocks=(1,)):
                t()
            for tt in range(4):
                for jb in range(2):
                    outproj_finish_thunk(tt, jb)()
            emit_norm_head(10, h2["yD10"], 1, 2)
            emit_norm_head(11, h2["yD11"], 1, 2)
            for tt in range(4):
                emit_out_dma(tt)
            for tt in range(4, TT):
                for jb in range(2):
                    outproj_finish_thunk(tt, jb)()
                emit_out_dma(tt)


def build_program(loop=1):
    nc = bacc.Bacc("TRN2", target_bir_lowering=False, debug=False)
    x_d = nc.dram_tensor("xT", [C, T], BF16, kind="ExternalInput").ap()
    wa_d = nc.dram_tensor("WaT", [C, 3 * C], BF16, kind="ExternalInput").ap()
    ba_d = nc.dram_tensor("b_attn", [3 * C], F32, kind="ExternalInput").ap()
    wp_d = nc.dram_tensor("WpT", [C, C], BF16, kind="ExternalInput").ap()
    bp_d = nc.dram_tensor("b_proj", [C], F32, kind="ExternalInput").ap()
    y_d = nc.dram_tensor("y", [T, C], F32, kind="ExternalOutput").ap()

    with tile.TileContext(nc) as tc, ExitStack() as ctx:
        const = ctx.enter_context(tc.tile_pool(name="const", bufs=1))
        persist = ctx.enter_context(tc.tile_pool(name="persist", bufs=1))
        nat = ctx.enter_context(tc.tile_pool(name="nat", bufs=6))
        work = ctx.enter_context(tc.tile_pool(name="work", bufs=2))
        ptp = ctx.enter_context(tc.tile_pool(name="ptp", bufs=4))
        dram = ctx.enter_context(tc.tile_pool(name="dram", bufs=2, space="DRAM"))
        psS = ctx.enter_context(tc.tile_pool(name="psS", bufs=2, space="PSUM"))
        psP = ctx.enter_context(tc.tile_pool(name="psP", bufs=3, space="PSUM"))
        pools = (const, persist, nat, work, ptp, dram, psS, psP)

        cst = emit_consts(nc, tc, const, ba_d, bp_d)
        V0 = persist.tile([P, TT, 12, 65], BF16, tag="Vaug")
        nc.gpsimd.memset(V0[:, :, :, 64:65], 1.0)
        loop_cm = tc.For_i(0, loop, 1) if loop > 1 else contextlib.nullcontext()
        with loop_cm:
            emit_body(nc, tc, pools, cst, x_d, wa_d, wp_d, y_d)

    nc.compile()
    return nc


_CACHED_NC = None


def prep_in_maps(x, W_attn, b_attn, W_proj, b_proj):
    import ml_dtypes
    bf16 = ml_dtypes.bfloat16
    B = x.shape[0]
    assert B == N_CORES
    WaT = np.ascontiguousarray(np.asarray(W_attn, dtype=np.float32).T.astype(bf16))
    WpT = np.ascontiguousarray(np.asarray(W_proj, dtype=np.float32).T.astype(bf16))
    ba = np.asarray(b_attn, dtype=np.float32)
    bp = np.asarray(b_proj, dtype=np.float32)
    return [
        {
            "xT": np.ascontiguousarray(
                np.asarray(x[b], dtype=np.float32).T.astype(bf16)),
            "WaT": WaT,
            "b_attn": ba,
            "WpT": WpT,
            "b_proj": bp,
        }
        for b in range(B)
    ]


def kernel(x, W_attn, b_attn, W_proj, b_proj):
    from concourse.bass_utils import run_bass_kernel_spmd

    global _CACHED_NC
    if _CACHED_NC is None:
        _CACHED_NC = build_program(loop=1)
    nc = _CACHED_NC

    in_maps = prep_in_maps(x, W_attn, b_attn, W_proj, b_proj)
    res = run_bass_kernel_spmd(nc, in_maps, list(range(N_CORES)))
    return np.stack([res.results[b]["y"] for b in range(N_CORES)], axis=0)
